# revision 9
# baseline (speedup 1.0000x reference)
"""Trainium2 Bass kernel for nn_CrossAttenHead (YOSO-style cross-attention head).

Data-parallel over batch B=8 across 8 NeuronCores (1 sample/core).
Per core:
  stage A: hard-mask pooling  f = hard @ feat^T          (fp16 matmuls, f32 psum)
  stage B: proposal head: dysep x2, MHA(8 heads), FFN, cls/mask towers
           (f32 elementwise/LN, fp16 matmul operands)
  stage C: new_mask_preds = mask_kernels @ feat          (fp16, f32 psum)

Host side packs inputs into partition-major layouts (transposes/casts) so the
device does zero large transposes. The sigmoid(x)>0.5 hard threshold of the
reference (jax f32 CPU semantics, cutoff T0) is replicated exactly: mask values
are pre-scaled by 8192, cast to fp16, and any element whose fp16 comparison
against T0*8192 would disagree with the f32 comparison is nudged across the
boundary at pack time.
"""

import sys

for _p in ("/opt/trn_rl_repo", "/opt/pypackages"):
    if _p not in sys.path:
        sys.path.append(_p)

import math
import numpy as np

HID = 256
NPROP = 100
K1 = 3
NCLS = 80
NHEAD = 8
HW = 16384
B = 8
N_CORES = 8
DH = HID // NHEAD  # 32

# largest f32 x with jax-f32(cpu) sigmoid(x) <= 0.5  (reference hard threshold)
T0 = np.float32(8.940697e-08)
MASK_SCALE = np.float32(8192.0)
STH = float(np.float32(T0 * MASK_SCALE))

F16 = np.float16
_PROG = None
TRACE = False
LAST_EXEC_NS = None


# ---------------------------------------------------------------------------
# layouts shared by host packing and device program
# ---------------------------------------------------------------------------
def _wlayout():
    # weight pack: k-chunk-major blocks [128, N] along free dim, fp16
    specs = [
        ("fW", 256, NPROP + K1),
        ("kW", 256, NPROP + K1),
        ("attn_in_w", 256, 768),
        ("attn_out_w", 256, 256),
        ("ffn_w1", 256, 2048),
        ("ffn_w2", 2048, 256),
        ("cls_w", 256, 256),
        ("fc_cls_w", 256, NCLS + 1),
        ("mask_w0", 256, 256),
        ("mask_w1", 256, 256),
        ("mask_w2", 256, 256),
        ("fc_mask_w", 256, 256),
    ]
    off = {}
    cur = 0
    for name, k, n in specs:
        off[name] = (cur, k, n)
        cur += (k // 128) * n
    return off, cur


def _vlayout():
    # replicated per-free f32 vectors
    names = [
        ("fb3", 3),
        ("fng", 256), ("fnb", 256),
        ("f_norm_g", 256), ("f_norm_b", 256),
        ("kng", 256), ("knb", 256),
        ("k_norm_g", 256), ("k_norm_b", 256),
        ("s_norm_g", 256), ("s_norm_b", 256),
        ("ffn_norm_g", 256), ("ffn_norm_b", 256),
        ("cls_ln_g", 256), ("cls_ln_b", 256),
        ("mask_ln_g0", 256), ("mask_ln_b0", 256),
        ("mask_ln_g1", 256), ("mask_ln_b1", 256),
        ("mask_ln_g2", 256), ("mask_ln_b2", 256),
        ("attn_in_b_v", 256),
        ("attn_out_b", 256),
        ("ffn_b2", 256),
        ("fc_cls_b", NCLS + 1),
    ]
    off = {}
    cur = 0
    for name, n in names:
        off[name] = cur
        cur += n
    return off, cur


# btpack columns (per-partition f32 biases for transposed-layout adds)
BT_ATTN_IN = 0      # 6 cols: attn_in_b[j*128+p] (q,k only used for j<4; all 6 packed)
BT_B1 = 6           # 16 cols: ffn_b1[j*128+p]
BT_FC_MASK = 22     # 2 cols: fc_mask_b[j*128+p]
BT_FB_PW = 24       # 1 col: fW bias for pointwise part: fb[3+p] (p<100)
BT_KB_PW = 25       # 1 col: kb[3+p]
BT_COLS = 26

WOFF, WCOLS = _wlayout()
VOFF, VCOLS = _vlayout()


# ---------------------------------------------------------------------------
# device program
# ---------------------------------------------------------------------------
def _build_program():
    import concourse.bacc as bacc
    import concourse.mybir as mybir
    import concourse.tile as tile
    from concourse.masks import make_identity

    f32 = mybir.dt.float32
    f16 = mybir.dt.float16
    Alu = mybir.AluOpType
    Act = mybir.ActivationFunctionType

    nc = bacc.Bacc("TRN2", target_bir_lowering=False, debug=False, num_devices=N_CORES)

    # --- dram tensors ---
    d_featT = nc.dram_tensor("featT", [128, 128 * 256], f16, kind="ExternalInput").ap()
    d_featN = nc.dram_tensor("featN", [128, 2 * HW], f16, kind="ExternalInput").ap()
    d_maskT = nc.dram_tensor("maskT", [128, 128 * NPROP], f16, kind="ExternalInput").ap()
    d_pkT = nc.dram_tensor("pkT", [128, 2 * NPROP], f16, kind="ExternalInput").ap()
    d_wpack = nc.dram_tensor("wpack", [128, WCOLS], f16, kind="ExternalInput").ap()
    d_vpack = nc.dram_tensor("vpack", [128, VCOLS], f16, kind="ExternalInput").ap()
    d_btpack = nc.dram_tensor("btpack", [128, BT_COLS], f32, kind="ExternalInput").ap()
    d_cls = nc.dram_tensor("cls_o", [NPROP, NCLS + 1], f32, kind="ExternalOutput").ap()
    d_obj = nc.dram_tensor("obj_o", [NPROP, HID], f32, kind="ExternalOutput").ap()
    d_nmp = nc.dram_tensor("nmp_o", [NPROP, HW], f16, kind="ExternalOutput").ap()

    NSLAB = 8          # pooling slabs
    TPS = 128 // NSLAB  # 16 chunks per slab

    with tile.TileContext(nc) as tc:
        import contextlib
        ctx = contextlib.ExitStack()
        consts = ctx.enter_context(tc.tile_pool(name="consts", bufs=1))
        mslab_p = ctx.enter_context(tc.tile_pool(name="mslab", bufs=2))
        hslab_p = ctx.enter_context(tc.tile_pool(name="hslab", bufs=2))
        fslab_p = ctx.enter_context(tc.tile_pool(name="fslab", bufs=2))
        nmout_p = ctx.enter_context(tc.tile_pool(name="nmout", bufs=2))
        act_p = ctx.enter_context(tc.tile_pool(name="act", bufs=2))
        xt_p = ctx.enter_context(tc.tile_pool(name="xt", bufs=4))
        sm_p = ctx.enter_context(tc.tile_pool(name="sm", bufs=2))
        st_p = ctx.enter_context(tc.tile_pool(name="st", bufs=4))
        ps_acc = ctx.enter_context(tc.tile_pool(name="ps_acc", bufs=2, space="PSUM"))
        ps_col = ctx.enter_context(tc.tile_pool(name="ps_col", bufs=3, space="PSUM"))
        ps_tp = ctx.enter_context(tc.tile_pool(name="ps_tp", bufs=2, space="PSUM"))

        # --- consts ---
        ident32 = consts.tile([128, 128], f32)
        make_identity(nc, ident32)
        ident16 = consts.tile([128, 128], f16)
        make_identity(nc, ident16)

        # const APs for activation() float biases (0.0 default, 1e-5 LN eps)
        zero_c = consts.tile([128, 1], f32)
        nc.vector.memset(zero_c, 0.0)
        eps_c = consts.tile([128, 1], f32)
        nc.vector.memset(eps_c, 1e-5)
        nc.const_aps.aps[(f32, 0.0)] = zero_c
        nc.const_aps.aps[(f32, 1e-5)] = eps_c

        wpack = consts.tile([128, WCOLS], f16)
        nc.scalar.dma_start(out=wpack, in_=d_wpack)
        btpack = consts.tile([128, BT_COLS], f32)
        nc.scalar.dma_start(out=btpack, in_=d_btpack)
        pkT = consts.tile([128, 2 * NPROP], f16)
        nc.scalar.dma_start(out=pkT, in_=d_pkT)
        vpack = consts.tile([128, VCOLS], f16)
        nc.scalar.dma_start(out=vpack, in_=d_vpack)
        featN = consts.tile([128, 2 * HW], f16)

        def W(name, kc, n0=None, n1=None):
            # slice of weight `name`, k-chunk kc, free cols [n0:n1]
            off, k, n = WOFF[name]
            base = off + kc * n
            if n0 is None:
                n0, n1 = 0, n
            return wpack[:, base + n0: base + n1]

        def V(name, n=256, p=NPROP):
            return vpack[:p, VOFF[name]: VOFF[name] + n]

        # ------------------------------------------------------------------
        # stage A: pooling  f[n, c] = sum_hw hard[n, hw] feat[c, hw]
        # ------------------------------------------------------------------
        psum_f = ps_acc.tile([NPROP, 256], f32, tag="acc")
        for s in range(NSLAB):
            ms = mslab_p.tile([128, TPS * NPROP], f16, tag="ms")
            nc.sync.dma_start(out=ms, in_=d_maskT[:, s * TPS * NPROP:(s + 1) * TPS * NPROP])
            fs = fslab_p.tile([128, TPS * 256], f16, tag="fs")
            nc.sync.dma_start(out=fs, in_=d_featT[:, s * TPS * 256:(s + 1) * TPS * 256])
            hs = hslab_p.tile([128, TPS * NPROP], f16, tag="hs")
            nc.vector.tensor_scalar(hs, ms, STH, None, Alu.is_gt)
            for t in range(TPS):
                g = s * TPS + t
                nc.tensor.matmul(
                    psum_f,
                    hs[:, t * NPROP:(t + 1) * NPROP],
                    fs[:, t * 256:(t + 1) * 256],
                    start=(g == 0),
                    stop=(g == 127),
                )

        # featN load: queued on sync after the pooling slabs
        for piece in range(4):
            sl = slice(piece * 8192, (piece + 1) * 8192)
            nc.sync.dma_start(out=featN[:, sl], in_=d_featN[:, sl])

        f_sb = act_p.tile([128, 256], f32, tag="resid")
        nc.vector.tensor_copy(f_sb[:NPROP, :], psum_f)

        # ------------------------------------------------------------------
        # helpers
        # ------------------------------------------------------------------
        def layernorm(x_ap, g_name, b_name, out_relu=False):
            """LN over free dim (256) of [NPROP, 256] f32 AP -> new act tile (f32)."""
            stats = st_p.tile([128, 6], f32, tag="stats")
            nc.vector.bn_stats(out=stats[:NPROP, :], in_=x_ap)
            mv = st_p.tile([128, 2], f32, tag="mv")
            nc.vector.bn_aggr(out=mv[:NPROP, :], in_=stats[:NPROP, :])
            std = st_p.tile([128, 1], f32, tag="std")
            nc.scalar.activation(out=std[:NPROP, :], in_=mv[:NPROP, 1:2], func=Act.Sqrt, bias=1e-5)
            rstd = st_p.tile([128, 1], f32, tag="rstd")
            nc.vector.reciprocal(out=rstd[:NPROP, :], in_=std[:NPROP, :])
            y = act_p.tile([128, 256], f32, tag="lnout")
            nc.vector.tensor_scalar(
                y[:NPROP, :], x_ap, mv[:NPROP, 0:1], rstd[:NPROP, 0:1],
                Alu.subtract, Alu.mult,
            )
            nc.vector.tensor_tensor(y[:NPROP, :], y[:NPROP, :], V(g_name), Alu.mult)
            if out_relu:
                tmp = act_p.tile([128, 256], f32, tag="lnout2")
                nc.vector.tensor_tensor(tmp[:NPROP, :], y[:NPROP, :], V(b_name), Alu.add)
                nc.scalar.activation(out=y[:NPROP, :], in_=tmp[:NPROP, :], func=Act.Relu)
            else:
                nc.vector.tensor_tensor(y[:NPROP, :], y[:NPROP, :], V(b_name), Alu.add)
            return y

        def transpose100(x_ap_f32, nch=2, out_f16=True):
            """[100, nch*128] f32 -> xT tile [128, nch*100] (fp16), xT[:, c*100+n]."""
            xT = xt_p.tile([128, nch * NPROP], f16 if out_f16 else f32, tag="xT")
            for c in range(nch):
                pt = ps_tp.tile([128, NPROP], f32, tag="tp")
                nc.tensor.transpose(
                    pt, x_ap_f32[:, c * 128:(c + 1) * 128], ident32[:NPROP, :NPROP]
                )
                nc.vector.tensor_copy(xT[:, c * NPROP:(c + 1) * NPROP], pt)
            return xT

        def dysep(value_sb, wname, bt_pw_col, g_name, b_name):
            """value [100,256] f32 tile -> LN(dynconv(value)) [100,256] f32 tile."""
            NW = NPROP + K1
            # dw = (pk @ W)[:, :3] + b[:3]
            ps_w3 = ps_col.tile([NPROP, K1], f32, tag="col")
            for kc in range(2):
                nc.tensor.matmul(
                    ps_w3, pkT[:, kc * NPROP:(kc + 1) * NPROP], W(wname, kc, 0, K1),
                    start=(kc == 0), stop=(kc == 1),
                )
            dw = st_p.tile([128, K1], f32, tag="dw")
            nc.vector.tensor_tensor(dw[:NPROP, :], ps_w3, V("fb3", K1), Alu.add)
            # pwT = (pk @ W[:, 3:])^T + bT   (weight cols sliced so psum starts at partition 0)
            ps_wT = ps_col.tile([NPROP, NPROP], f32, tag="col")
            for kc in range(2):
                nc.tensor.matmul(
                    ps_wT, W(wname, kc, K1, NW), pkT[:, kc * NPROP:(kc + 1) * NPROP],
                    start=(kc == 0), stop=(kc == 1),
                )
            pwT = sm_p.tile([128, NPROP], f16, tag="pwT")
            nc.scalar.activation(
                out=pwT[:NPROP, :], in_=ps_wT, func=Act.Identity,
                bias=btpack[:NPROP, bt_pw_col:bt_pw_col + 1],
            )
            # depth = relu(sum_k dw_k * pad(value)[:, k:k+256])
            fpad = act_p.tile([128, 258], f32, tag="fpad")
            nc.vector.memset(fpad[:NPROP, 0:1], 0.0)
            nc.vector.memset(fpad[:NPROP, 257:258], 0.0)
            nc.vector.tensor_copy(fpad[:NPROP, 1:257], value_sb[:NPROP, :])
            acc = act_p.tile([128, 256], f32, tag="dacc")
            nc.vector.tensor_scalar(
                acc[:NPROP, :], fpad[:NPROP, 0:256], dw[:NPROP, 0:1], None, Alu.mult
            )
            tmp = act_p.tile([128, 256], f32, tag="dtmp")
            nc.vector.tensor_scalar(
                tmp[:NPROP, :], fpad[:NPROP, 1:257], dw[:NPROP, 1:2], None, Alu.mult
            )
            nc.vector.tensor_tensor(acc[:NPROP, :], acc[:NPROP, :], tmp[:NPROP, :], Alu.add)
            nc.vector.tensor_scalar(
                tmp[:NPROP, :], fpad[:NPROP, 2:258], dw[:NPROP, 2:3], None, Alu.mult
            )
            nc.vector.tensor_tensor(acc[:NPROP, :], acc[:NPROP, :], tmp[:NPROP, :], Alu.add)
            depth16 = sm_p.tile([128, 256], f16, tag="depth16")
            nc.scalar.activation(out=depth16[:NPROP, :], in_=acc[:NPROP, :], func=Act.Relu)
            # point = pw @ depth  (contract proposals)
            ps_pt = ps_acc.tile([NPROP, 256], f32, tag="acc")
            nc.tensor.matmul(
                ps_pt, pwT[:NPROP, :], depth16[:NPROP, :], start=True, stop=True
            )
            return layernorm(ps_pt, g_name, b_name)

        def resid_ln(res_sb, d_ap, g_name, b_name, extra_vec=None):
            """LN(res + d (+ extra_vec)) -> f32 tile; d_ap may be psum."""
            x = act_p.tile([128, 256], f32, tag="resid")
            if extra_vec is not None:
                nc.vector.tensor_tensor(x[:NPROP, :], d_ap, V(extra_vec), Alu.add)
                nc.vector.tensor_tensor(x[:NPROP, :], x[:NPROP, :], res_sb[:NPROP, :], Alu.add)
            else:
                nc.vector.tensor_tensor(x[:NPROP, :], d_ap, res_sb[:NPROP, :], Alu.add)
            return layernorm(x[:NPROP, :], g_name, b_name)

        # ------------------------------------------------------------------
        # stage B: proposal head
        # ------------------------------------------------------------------
        d1 = dysep(f_sb, "fW", BT_FB_PW, "fng", "fnb")
        f1 = resid_ln(f_sb, d1[:NPROP, :], "f_norm_g", "f_norm_b")
        d2 = dysep(f1, "kW", BT_KB_PW, "kng", "knb")
        kk = resid_ln(f1, d2[:NPROP, :], "k_norm_g", "k_norm_b")

        # --- MHA ---
        kkT = transpose100(kk[:NPROP, :])
        qkvT = sm_p.tile([128, 4 * NPROP], f16, tag="qkvT")  # q,k transposed (4 chunks)
        for j in range(4):
            ps_q = ps_col.tile([128, NPROP], f32, tag="col")
            for kc in range(2):
                nc.tensor.matmul(
                    ps_q, W("attn_in_w", kc, j * 128, (j + 1) * 128),
                    kkT[:, kc * NPROP:(kc + 1) * NPROP],
                    start=(kc == 0), stop=(kc == 1),
                )
            nc.scalar.activation(
                out=qkvT[:, j * NPROP:(j + 1) * NPROP], in_=ps_q, func=Act.Identity,
                bias=btpack[:, BT_ATTN_IN + j: BT_ATTN_IN + j + 1],
            )
        # v natural [100, 256]
        ps_v = ps_acc.tile([NPROP, 256], f32, tag="acc")
        for kc in range(2):
            nc.tensor.matmul(
                ps_v, kkT[:, kc * NPROP:(kc + 1) * NPROP], W("attn_in_w", kc, 512, 768),
                start=(kc == 0), stop=(kc == 1),
            )
        v16 = sm_p.tile([128, 256], f16, tag="v16")
        vtmp = act_p.tile([128, 256], f32, tag="vtmp")
        nc.vector.tensor_tensor(vtmp[:NPROP, :], ps_v, V("attn_in_b_v"), Alu.add)
        nc.vector.tensor_copy(v16[:NPROP, :], vtmp[:NPROP, :])

        den = st_p.tile([128, NHEAD], f32, tag="den")
        rec = st_p.tile([128, NHEAD], f32, tag="rec")
        ps_oT = []
        for _i in range(2):
            ps_oT_i = ps_col.tile([128, NPROP], f32, tag="col")
            ps_oT.append(ps_oT_i)
        for h in range(NHEAD):
            pb = (h % 4) * 32
            jq = h // 4
            ps_s = ps_tp.tile([NPROP, NPROP], f32, tag="tp")
            nc.tensor.matmul(
                ps_s,
                qkvT[pb:pb + 32, jq * NPROP:(jq + 1) * NPROP],
                qkvT[pb:pb + 32, (2 + jq) * NPROP:(3 + jq) * NPROP],
                start=True, stop=True,
                tile_position=(pb, 0),
            )
            e16 = sm_p.tile([128, NPROP], f16, tag="e16")
            nc.scalar.activation(
                out=e16[:NPROP, :], in_=ps_s, func=Act.Exp,
                scale=float(1.0 / math.sqrt(DH)),
                accum_out=den[:NPROP, h:h + 1],
            )
            nc.vector.reciprocal(out=rec[:NPROP, h:h + 1], in_=den[:NPROP, h:h + 1])
            en16 = sm_p.tile([128, NPROP], f16, tag="en16")
            nc.vector.tensor_scalar(
                en16[:NPROP, :], e16[:NPROP, :], rec[:NPROP, h:h + 1], None, Alu.mult
            )
            ps_t = ps_tp.tile([NPROP, NPROP], f16, tag="tp")
            nc.tensor.transpose(ps_t, en16[:NPROP, :NPROP], ident16[:NPROP, :NPROP])
            sT = sm_p.tile([128, NPROP], f16, tag="sT")
            nc.vector.tensor_copy(sT[:NPROP, :], ps_t)
            nc.tensor.matmul(
                ps_oT[jq][pb:pb + 32, :], v16[:NPROP, h * 32:(h + 1) * 32],
                sT[:NPROP, :], start=True, stop=True,
                tile_position=(0, pb),
            )
        oT = sm_p.tile([128, 2 * NPROP], f16, tag="oT")
        for jq in range(2):
            nc.vector.tensor_copy(oT[:, jq * NPROP:(jq + 1) * NPROP], ps_oT[jq])
        ps_kk2 = ps_acc.tile([NPROP, 256], f32, tag="acc")
        for kc in range(2):
            nc.tensor.matmul(
                ps_kk2, oT[:, kc * NPROP:(kc + 1) * NPROP], W("attn_out_w", kc),
                start=(kc == 0), stop=(kc == 1),
            )
        s2 = resid_ln(kk, ps_kk2, "s_norm_g", "s_norm_b", extra_vec="attn_out_b")

        # --- FFN ---
        s2T = transpose100(s2[:NPROP, :])
        hT = sm_p.tile([128, 16 * NPROP], f16, tag="hT")
        for j in range(16):
            ps_h = ps_col.tile([128, NPROP], f32, tag="col")
            for kc in range(2):
                nc.tensor.matmul(
                    ps_h, W("ffn_w1", kc, j * 128, (j + 1) * 128),
                    s2T[:, kc * NPROP:(kc + 1) * NPROP],
                    start=(kc == 0), stop=(kc == 1),
                )
            nc.scalar.activation(
                out=hT[:, j * NPROP:(j + 1) * NPROP], in_=ps_h, func=Act.Relu,
                bias=btpack[:, BT_B1 + j: BT_B1 + j + 1],
            )
        ps_r = ps_acc.tile([NPROP, 256], f32, tag="acc")
        for j in range(16):
            nc.tensor.matmul(
                ps_r, hT[:, j * NPROP:(j + 1) * NPROP], W("ffn_w2", j),
                start=(j == 0), stop=(j == 15),
            )
        obj2 = resid_ln(s2, ps_r, "ffn_norm_g", "ffn_norm_b", extra_vec="ffn_b2")
        nc.scalar.dma_start(out=d_obj, in_=obj2[:NPROP, :])

        # --- cls head ---
        objT = transpose100(obj2[:NPROP, :])
        ps_c = ps_acc.tile([NPROP, 256], f32, tag="acc")
        for kc in range(2):
            nc.tensor.matmul(
                ps_c, objT[:, kc * NPROP:(kc + 1) * NPROP], W("cls_w", kc),
                start=(kc == 0), stop=(kc == 1),
            )
        cls_feat = layernorm(ps_c, "cls_ln_g", "cls_ln_b", out_relu=True)
        clsT = transpose100(cls_feat[:NPROP, :])
        ps_cs = ps_col.tile([NPROP, NCLS + 1], f32, tag="col")
        for kc in range(2):
            nc.tensor.matmul(
                ps_cs, clsT[:, kc * NPROP:(kc + 1) * NPROP], W("fc_cls_w", kc),
                start=(kc == 0), stop=(kc == 1),
            )
        cls_sb = act_p.tile([128, NCLS + 1], f32, tag="cls")
        nc.vector.tensor_tensor(cls_sb[:NPROP, :], ps_cs, V("fc_cls_b", NCLS + 1), Alu.add)
        nc.scalar.dma_start(out=d_cls, in_=cls_sb[:NPROP, :])

        # --- mask tower ---
        mf = obj2
        for i in range(3):
            mfT = transpose100(mf[:NPROP, :])
            ps_m = ps_acc.tile([NPROP, 256], f32, tag="acc")
            for kc in range(2):
                nc.tensor.matmul(
                    ps_m, mfT[:, kc * NPROP:(kc + 1) * NPROP], W(f"mask_w{i}", kc),
                    start=(kc == 0), stop=(kc == 1),
                )
            mf = layernorm(ps_m, f"mask_ln_g{i}", f"mask_ln_b{i}", out_relu=True)
        mfT = transpose100(mf[:NPROP, :])
        mkT = sm_p.tile([128, 2 * NPROP], f16, tag="mkT")
        for mc in range(2):
            ps_mk = ps_col.tile([128, NPROP], f32, tag="col")
            for kc in range(2):
                nc.tensor.matmul(
                    ps_mk, W("fc_mask_w", kc, mc * 128, (mc + 1) * 128),
                    mfT[:, kc * NPROP:(kc + 1) * NPROP],
                    start=(kc == 0), stop=(kc == 1),
                )
            nc.scalar.activation(
                out=mkT[:, mc * NPROP:(mc + 1) * NPROP], in_=ps_mk, func=Act.Identity,
                bias=btpack[:, BT_FC_MASK + mc: BT_FC_MASK + mc + 1],
            )

        # ------------------------------------------------------------------
        # stage C: new_mask_preds = mask_kernels @ feat
        # ------------------------------------------------------------------
        NOUT = 512
        GPS = 4096 // NOUT  # psum groups per out slab
        for so in range(HW // 4096):
            nm = nmout_p.tile([128, 4096], f16, tag="nm")
            for gi in range(GPS):
                ps_n = ps_acc.tile([NPROP, NOUT], f32, tag="acc")
                hw0 = so * 4096 + gi * NOUT
                for kc in range(2):
                    nc.tensor.matmul(
                        ps_n, mkT[:, kc * NPROP:(kc + 1) * NPROP],
                        featN[:, kc * HW + hw0: kc * HW + hw0 + NOUT],
                        start=(kc == 0), stop=(kc == 1),
                    )
                dst = nm[:NPROP, gi * NOUT:(gi + 1) * NOUT]
                if gi % 2 == 0:
                    nc.vector.tensor_copy(dst, ps_n)
                else:
                    nc.scalar.copy(out=dst, in_=ps_n)
            nc.scalar.dma_start(
                out=d_nmp[:, so * 4096:(so + 1) * 4096], in_=nm[:NPROP, :]
            )
        ctx.close()

    nc.compile()
    return nc


# ---------------------------------------------------------------------------
# host packing
# ---------------------------------------------------------------------------
def _pack_weights(inputs):
    def blk(w):
        w = np.asarray(w, np.float32)
        k, n = w.shape
        return w.reshape(k // 128, 128, n).transpose(1, 0, 2).reshape(128, -1)

    parts = [
        blk(inputs["fW"]), blk(inputs["kW"]),
        blk(inputs["attn_in_w"]), blk(inputs["attn_out_w"]),
        blk(inputs["ffn_w1"]), blk(inputs["ffn_w2"]),
        blk(inputs["cls_w"]), blk(inputs["fc_cls_w"]),
        blk(np.asarray(inputs["mask_w"])[0]), blk(np.asarray(inputs["mask_w"])[1]),
        blk(np.asarray(inputs["mask_w"])[2]), blk(inputs["fc_mask_w"]),
    ]
    wp = np.concatenate(parts, axis=1)
    assert wp.shape == (128, WCOLS), wp.shape
    return np.ascontiguousarray(wp.astype(F16))


def _pack_vectors(inputs):
    v = np.zeros(VCOLS, np.float32)

    def put(name, arr):
        a = np.asarray(arr, np.float32).ravel()
        v[VOFF[name]: VOFF[name] + a.size] = a

    put("fb3", np.asarray(inputs["fb"], np.float32)[:3])
    for nm in ["fng", "fnb", "f_norm_g", "f_norm_b", "kng", "knb", "k_norm_g",
               "k_norm_b", "s_norm_g", "s_norm_b", "ffn_norm_g", "ffn_norm_b",
               "cls_ln_g", "cls_ln_b"]:
        put(nm, inputs[nm])
    mg = np.asarray(inputs["mask_ln_g"], np.float32)
    mb = np.asarray(inputs["mask_ln_b"], np.float32)
    for i in range(3):
        put(f"mask_ln_g{i}", mg[i])
        put(f"mask_ln_b{i}", mb[i])
    put("attn_in_b_v", np.asarray(inputs["attn_in_b"], np.float32)[512:768])
    put("attn_out_b", inputs["attn_out_b"])
    put("ffn_b2", inputs["ffn_b2"])
    put("fc_cls_b", inputs["fc_cls_b"])
    return np.ascontiguousarray(np.broadcast_to(v[None, :], (128, VCOLS)).astype(F16))


def _pack_bt(inputs):
    bt = np.zeros((128, BT_COLS), np.float32)
    aib = np.asarray(inputs["attn_in_b"], np.float32)
    for j in range(6):
        bt[:, BT_ATTN_IN + j] = aib[j * 128:(j + 1) * 128]
    b1 = np.asarray(inputs["ffn_b1"], np.float32)
    for j in range(16):
        bt[:, BT_B1 + j] = b1[j * 128:(j + 1) * 128]
    fmb = np.asarray(inputs["fc_mask_b"], np.float32)
    for j in range(2):
        bt[:, BT_FC_MASK + j] = fmb[j * 128:(j + 1) * 128]
    bt[:NPROP, BT_FB_PW] = np.asarray(inputs["fb"], np.float32)[K1:]
    bt[:NPROP, BT_KB_PW] = np.asarray(inputs["kb"], np.float32)[K1:]
    return np.ascontiguousarray(bt)


def _pack_mask(mask_b):
    """[100, HW] f32 -> fp16 [128, 128*100], scaled by 8192 and boundary-patched."""
    m = np.asarray(mask_b, np.float32)
    want = m > T0
    m16 = (m * MASK_SCALE).astype(F16)
    dev = m16.astype(np.float32) > STH
    bad_on = want & ~dev
    bad_off = dev & ~want
    if bad_on.any():
        m16[bad_on] = F16(0.01)
    if bad_off.any():
        m16[bad_off] = F16(0.0)
    out = np.zeros((128, 128 * NPROP), F16)
    out.reshape(128, 128, NPROP)[:] = m16.reshape(NPROP, 128, 128).transpose(2, 1, 0)
    return out


def kernel(**inputs):
    global _PROG
    from concourse import bass_utils

    if _PROG is None:
        _PROG = _build_program()
    nc = _PROG

    feats = np.asarray(inputs["features"], np.float32)
    pks = np.asarray(inputs["proposal_kernels"], np.float32)
    masks = np.asarray(inputs["mask_preds"], np.float32)
    train_flag = int(np.asarray(inputs["train_flag"]))

    wpack = _pack_weights(inputs)
    vpack = _pack_vectors(inputs)
    btpack = _pack_bt(inputs)

    in_maps = []
    for b in range(B):
        feat = feats[b].reshape(256, HW)
        f16feat = feat.astype(F16)
        featT = np.ascontiguousarray(
            f16feat.reshape(256, 128, 128).transpose(2, 1, 0).reshape(128, 128 * 256)
        )
        featN = np.ascontiguousarray(
            f16feat.reshape(2, 128, HW).transpose(1, 0, 2).reshape(128, 2 * HW)
        )
        maskT = _pack_mask(masks[b].reshape(NPROP, HW))
        pkT = np.ascontiguousarray(
            pks[b].astype(F16).T.reshape(2, 128, NPROP).transpose(1, 0, 2).reshape(128, 2 * NPROP)
        )
        in_maps.append({
            "featT": featT, "featN": featN, "maskT": maskT, "pkT": pkT,
            "wpack": wpack, "vpack": vpack, "btpack": btpack,
        })

    global LAST_EXEC_NS
    res = bass_utils.run_bass_kernel_spmd(
        nc, in_maps, core_ids=list(range(N_CORES)), trace=TRACE
    )
    LAST_EXEC_NS = res.exec_time_ns

    cls = np.stack([res.results[b]["cls_o"] for b in range(B)]).astype(np.float32)
    nmp = np.stack([res.results[b]["nmp_o"] for b in range(B)]).astype(np.float32)
    obj = np.stack([res.results[b]["obj_o"] for b in range(B)]).astype(np.float32)

    cls_score = cls.reshape(B, NPROP, NCLS + 1) if train_flag else None
    new_mask_preds = nmp.reshape(B, NPROP, 128, 128)
    obj_out = obj.reshape(B, NPROP, HID, 1, 1)
    return (cls_score, new_mask_preds, obj_out)


if __name__ == "__main__":
    _PROG = _build_program()
    print("program built ok")
